# revision 3
# baseline (speedup 1.0000x reference)
"""MoE SwiGLU MLP (top-2 of 8 experts) on 8 Trainium2 NeuronCores.

Strategy: expert-parallel with token routing. The router (a 1024x8 matmul +
softmax + top-2) is tiny, so it runs on the host as part of sharding. Each
core is assigned one expert and receives only the tokens routed to it
(gathered + transposed on the host into PE-friendly layouts). On-device each
core runs a dense SwiGLU MLP over its [C, 1024] token slab with f32r
(FP22) matmuls, scales by the renormalized router weight, and the host
scatter-adds the two per-token expert contributions back into the full
[2, 2048, 1024] output.
"""

import time

import numpy as np

B, S, D, M, E, TOP_K = 2, 2048, 1024, 2048, 8, 2
N = B * S
P = 128
KD = D // P   # 8  k-subtiles over the d contraction
KM = M // P   # 16 k-subtiles over the m contraction
MC = M // P   # 16 m-chunks (phase A output partitions)
DC = D // P   # 8  d-chunks (phase B output partitions)
TCHUNK = 512

_runner_cache: dict[int, object] = {}
LAST_RUN: dict = {}


def _build_bass(C: int, R: int = 1):
    import contextlib

    import concourse.bacc as bacc
    import concourse.mybir as mybir
    import concourse.tile as tile

    f32 = mybir.dt.float32
    f32r = mybir.dt.float32r

    nc = bacc.Bacc("TRN2", target_bir_lowering=False, debug=False, num_devices=8)

    xt = nc.dram_tensor("xt", [P, KD, C], f32r, kind="ExternalInput")
    wg = nc.dram_tensor("wg", [MC, P, KD, P], f32r, kind="ExternalInput")
    wu = nc.dram_tensor("wu", [MC, P, KD, P], f32r, kind="ExternalInput")
    wo = nc.dram_tensor("wo", [DC, P, KM, P], f32r, kind="ExternalInput")
    wrep = nc.dram_tensor("wrep", [P, C], f32, kind="ExternalInput")
    out = nc.dram_tensor("out", [DC, P, C], f32, kind="ExternalOutput")

    tch = [(i * TCHUNK, min(TCHUNK, C - i * TCHUNK)) for i in range((C + TCHUNK - 1) // TCHUNK)]

    with tile.TileContext(nc) as tc:
        with (
            tc.For_i(0, R, 1) if R > 1 else contextlib.nullcontext(),
            tc.tile_pool(name="big", bufs=1) as big,
            tc.tile_pool(name="wpool", bufs=3) as wpool,
            tc.tile_pool(name="tmp", bufs=3) as tmp,
            tc.tile_pool(name="psg_pool", bufs=4, space="PSUM") as psg_pool,
            tc.tile_pool(name="psu_pool", bufs=4, space="PSUM") as psu_pool,
        ):
            xt_sb = big.tile([P, KD, C], f32r)
            nc.sync.dma_start(xt_sb[:], xt[:])
            wrep_sb = big.tile([P, C], f32)
            nc.sync.dma_start(wrep_sb[:], wrep[:])
            h_sb = big.tile([P, KM, C], f32r)

            # ---- phase A: hT[m, t] = silu(gateT) * upT over 16 m-chunks ----
            # k-outer / t-inner: consecutive matmuls share the stationary
            # weight chunk, which measures ~10% faster than t-outer (the
            # redundant LDWEIGHTS pipeline much better).
            for mc in range(MC):
                wg_sb = wpool.tile([P, KD, P], f32r, tag="wg")
                nc.sync.dma_start(wg_sb[:], wg[mc])
                wu_sb = wpool.tile([P, KD, P], f32r, tag="wu")
                nc.sync.dma_start(wu_sb[:], wu[mc])
                ps_gs = [psg_pool.tile([P, TCHUNK], f32, tag="psg", name=f"psg{i}")
                         for i in range(len(tch))]
                ps_us = [psu_pool.tile([P, TCHUNK], f32, tag="psu", name=f"psu{i}")
                         for i in range(len(tch))]
                for k in range(KD):
                    for i, (t0, tw) in enumerate(tch):
                        nc.tensor.matmul(
                            ps_gs[i][:, :tw], wg_sb[:, k, :],
                            xt_sb[:, k, t0 : t0 + tw],
                            start=(k == 0), stop=(k == KD - 1),
                        )
                for k in range(KD):
                    for i, (t0, tw) in enumerate(tch):
                        nc.tensor.matmul(
                            ps_us[i][:, :tw], wu_sb[:, k, :],
                            xt_sb[:, k, t0 : t0 + tw],
                            start=(k == 0), stop=(k == KD - 1),
                        )
                for i, (t0, tw) in enumerate(tch):
                    g_sb = tmp.tile([P, TCHUNK], f32, tag="g")
                    nc.scalar.activation(
                        g_sb[:, :tw], ps_gs[i][:, :tw],
                        func=mybir.ActivationFunctionType.Silu,
                    )
                    nc.vector.tensor_mul(
                        h_sb[:, mc, t0 : t0 + tw], g_sb[:, :tw], ps_us[i][:, :tw]
                    )

            # ---- phase B: yT[d, t] = (hT.T @ Wo).T * w[t] over 8 d-chunks ----
            # psum tiles reuse the phase-A "psg" slots (phases are sequential)
            for dc in range(DC):
                wo_sb = wpool.tile([P, KM, P], f32r, tag="wo")
                nc.sync.dma_start(wo_sb[:], wo[dc])
                ps_ys = [psg_pool.tile([P, TCHUNK], f32, tag="psg", name=f"psy{i}")
                         for i in range(len(tch))]
                for k in range(KM):
                    for i, (t0, tw) in enumerate(tch):
                        nc.tensor.matmul(
                            ps_ys[i][:, :tw], wo_sb[:, k, :],
                            h_sb[:, k, t0 : t0 + tw],
                            start=(k == 0), stop=(k == KM - 1),
                        )
                for i, (t0, tw) in enumerate(tch):
                    o_sb = tmp.tile([P, TCHUNK], f32, tag="o")
                    nc.vector.tensor_mul(
                        o_sb[:, :tw], ps_ys[i][:, :tw], wrep_sb[:, t0 : t0 + tw]
                    )
                    nc.sync.dma_start(out[dc, :, t0 : t0 + tw], o_sb[:, :tw])

    nc.compile()
    return nc


class _Runner:
    """Persistent jitted SPMD executor (mirrors bass2jax.run_bass_via_pjrt,
    but reusable across calls so repeated runs skip retrace/recompile)."""

    def __init__(self, nc, n_cores=8):
        import jax
        from jax.sharding import Mesh, PartitionSpec
        from jax.experimental.shard_map import shard_map
        import concourse.mybir as mybir
        from concourse import bass2jax

        bass2jax.install_neuronx_cc_hook()
        self.jax = jax
        self.n_cores = n_cores

        partition_name = (
            nc.partition_id_tensor.name if nc.partition_id_tensor else None
        )
        in_names, out_names, out_avals, zero_outs = [], [], [], []
        for alloc in nc.m.functions[0].allocations:
            if not isinstance(alloc, mybir.MemoryLocationSet):
                continue
            name = alloc.memorylocations[0].name
            if alloc.kind == "ExternalInput":
                if name != partition_name:
                    in_names.append(name)
            elif alloc.kind == "ExternalOutput":
                shape = tuple(alloc.tensor_shape)
                dtype = mybir.dt.np(alloc.dtype)
                out_names.append(name)
                out_avals.append(jax.core.ShapedArray(shape, dtype))
                zero_outs.append(np.zeros(shape, dtype))
        self.in_names = list(in_names)
        self.out_names = list(out_names)
        self.out_avals = out_avals
        n_params = len(in_names)
        all_in_names = in_names + out_names
        if partition_name is not None:
            all_in_names = all_in_names + [partition_name]

        def _call_once(operands):
            return bass2jax._bass_exec_p.bind(
                *operands,
                out_avals=tuple(out_avals),
                in_names=tuple(all_in_names),
                out_names=tuple(out_names),
                lowering_input_output_aliases=(),
                sim_require_finite=True,
                sim_require_nnan=True,
                nc=nc,
            )

        def _make_body(reps):
            def _body(*args):
                operands = list(args)
                if partition_name is not None:
                    operands.append(bass2jax.partition_id_tensor())
                outs = _call_once(operands)
                for _ in range(reps - 1):
                    outs = _call_once(operands)
                return tuple(outs)

            return _body

        devices = jax.devices()[:n_cores]
        assert len(devices) == n_cores
        mesh = Mesh(np.asarray(devices), ("core",))
        in_specs = (PartitionSpec("core"),) * (n_params + len(out_names))
        out_specs = (PartitionSpec("core"),) * len(out_names)

        def _jit(reps):
            return jax.jit(
                shard_map(_make_body(reps), mesh=mesh, in_specs=in_specs,
                          out_specs=out_specs, check_rep=False),
                keep_unused=True,
            )

        self._fns = {}
        self._jit = _jit
        self._fn = self.get_fn(1)
        self._zero_concat = [
            np.zeros((n_cores * z.shape[0], *z.shape[1:]), z.dtype) for z in zero_outs
        ]

    def run(self, in_maps):
        concat_in = [
            np.concatenate([np.asarray(m[name]) for m in in_maps], axis=0)
            for name in self.in_names
        ]
        t0 = time.time()
        out_arrs = self._fn(*concat_in, *self._zero_concat)
        out_arrs = [np.asarray(a) for a in out_arrs]
        LAST_RUN["run_s"] = time.time() - t0
        return [
            {
                name: out_arrs[i].reshape(self.n_cores, *self.out_avals[i].shape)[c]
                for i, name in enumerate(self.out_names)
            }
            for c in range(self.n_cores)
        ]

    def get_fn(self, reps):
        if reps not in self._fns:
            self._fns[reps] = self._jit(reps)
        return self._fns[reps]

    def _time_fn(self, fn, dev_in, dev_zero, iters):
        jax = self.jax
        r = fn(*dev_in, *dev_zero)  # warmup / compile
        jax.block_until_ready(r)
        times = []
        for _ in range(iters):
            t0 = time.perf_counter()
            r = fn(*dev_in, *dev_zero)
            jax.block_until_ready(r)
            times.append(time.perf_counter() - t0)
        return min(times)

    def bench(self, in_maps, iters=3, reps=8):
        """Time reps-in-one-launch vs 1; slope isolates per-NEFF-exec time
        from axon dispatch overhead."""
        concat_in = [
            np.concatenate([np.asarray(m[name]) for m in in_maps], axis=0)
            for name in self.in_names
        ]
        jax = self.jax
        dev_in = [jax.device_put(a) for a in concat_in]
        dev_zero = [jax.device_put(a) for a in self._zero_concat]
        t1 = self._time_fn(self.get_fn(1), dev_in, dev_zero, iters)
        tn = self._time_fn(self.get_fn(reps), dev_in, dev_zero, iters)
        per_exec = (tn - t1) / (reps - 1)
        return {"t1_s": t1, "tn_s": tn, "reps": reps, "per_exec_s": per_exec}


def _route(residual: np.ndarray, W_router: np.ndarray):
    """Host router: softmax over experts, top-2 (desc, ties -> lower idx),
    renormalize. Returns per-expert (token_ids, weights)."""
    X = residual.reshape(N, D).astype(np.float32)
    logits = X @ W_router.astype(np.float32)
    mx = logits.max(axis=-1, keepdims=True)
    e = np.exp(logits - mx)
    probs = e / e.sum(axis=-1, keepdims=True)
    order = np.argsort(-probs, axis=-1, kind="stable")[:, :TOP_K]       # [N, 2]
    vals = np.take_along_axis(probs, order, axis=-1)                     # [N, 2]
    wts = vals / (vals.sum(axis=-1, keepdims=True) + 1e-8)
    ids, ws = [], []
    for ex in range(E):
        hit = order == ex                                                # [N, 2]
        sel = np.nonzero(hit.any(axis=-1))[0]
        w_tok = np.where(hit[sel, 0], wts[sel, 0], wts[sel, 1]).astype(np.float32)
        ids.append(sel)
        ws.append(w_tok)
    return X, ids, ws


def kernel(
    residual, W_router, W_gate, b_gate, W_up, b_up, W_out, b_out
) -> np.ndarray:
    # NOTE: b_gate/b_up/b_out have fill=zeros in the problem spec and are
    # therefore not applied on-device.
    t_host0 = time.time()
    X, ids, ws = _route(np.asarray(residual), np.asarray(W_router))
    counts = [len(s) for s in ids]
    C = max(P, ((max(counts) + P - 1) // P) * P)

    W_gate = np.ascontiguousarray(np.asarray(W_gate, dtype=np.float32))
    W_up = np.ascontiguousarray(np.asarray(W_up, dtype=np.float32))
    W_out = np.ascontiguousarray(np.asarray(W_out, dtype=np.float32))

    in_maps = []
    for ex in range(E):
        n_e = counts[ex]
        xt = np.zeros((P, KD, C), np.float32)
        xt[:, :, :n_e] = X[ids[ex]].T.reshape(KD, P, n_e).transpose(1, 0, 2)
        wrep = np.zeros((P, C), np.float32)
        wrep[:, :n_e] = ws[ex][None, :]
        in_maps.append(
            {
                "xt": xt,
                "wg": np.ascontiguousarray(
                    W_gate[ex].reshape(KD, P, MC, P).transpose(2, 1, 0, 3)
                ),
                "wu": np.ascontiguousarray(
                    W_up[ex].reshape(KD, P, MC, P).transpose(2, 1, 0, 3)
                ),
                "wo": np.ascontiguousarray(
                    W_out[ex].reshape(KM, P, DC, P).transpose(2, 1, 0, 3)
                ),
                "wrep": wrep,
            }
        )
    LAST_RUN["host_prep_s"] = time.time() - t_host0
    LAST_RUN["C"] = C
    LAST_RUN["counts"] = counts
    LAST_RUN["in_maps"] = in_maps

    if C not in _runner_cache:
        t0 = time.time()
        nc = _build_bass(C)
        LAST_RUN["build_s"] = time.time() - t0
        _runner_cache[C] = _Runner(nc)
    runner = _runner_cache[C]
    results = runner.run(in_maps)

    res = np.zeros((N, D), np.float32)
    for ex in range(E):
        n_e = counts[ex]
        y = results[ex]["out"].reshape(D, C)[:, :n_e]                    # [D, n_e]
        res[ids[ex]] += y.T
    return res.reshape(B, S, D)


def get_runner(C: int):
    return _runner_cache.get(C)



# revision 4
# speedup vs baseline: 1.1395x; 1.1395x over previous
"""MoE SwiGLU MLP (top-2 of 8 experts) on 8 Trainium2 NeuronCores.

Strategy: expert-parallel with token routing. The router (a 1024x8 matmul +
softmax + top-2) is tiny, so it runs on the host as part of sharding. Each
core is assigned one expert and receives only the tokens routed to it
(gathered + transposed on the host into PE-friendly layouts, converted to
bf16). On-device each core runs a dense SwiGLU MLP over its [C, 1024] token
slab with bf16 matmuls (f32 PSUM accumulation), scales by the renormalized
router weight, and the host scatter-adds the two per-token expert
contributions back into the full [2, 2048, 1024] output.

bf16 matters twice: the PE streams 1 column/cycle at any chunk width (f32r
drops to 1/4 rate below 256 columns, which made the old 128-wide tail chunk
cost 4x), and weight DMA bytes halve so the per-expert 12.6MB streams well
under the compute time.
"""

import time

import numpy as np

B, S, D, M, E, TOP_K = 2, 2048, 1024, 2048, 8, 2
N = B * S
P = 128
KD = D // P   # 8  k-subtiles over the d contraction
KM = M // P   # 16 k-subtiles over the m contraction
MC = M // P   # 16 m-chunks (phase A output partitions)
DC = D // P   # 8  d-chunks (phase B output partitions)
TCHUNK = 512

_runner_cache: dict[int, object] = {}
LAST_RUN: dict = {}


def _build_bass(C: int, R: int = 1):
    import contextlib

    import concourse.bacc as bacc
    import concourse.mybir as mybir
    import concourse.tile as tile

    f32 = mybir.dt.float32
    bf16 = mybir.dt.bfloat16

    nc = bacc.Bacc("TRN2", target_bir_lowering=False, debug=False, num_devices=8)

    xt = nc.dram_tensor("xt", [P, KD, C], bf16, kind="ExternalInput")
    wg = nc.dram_tensor("wg", [MC, P, KD, P], bf16, kind="ExternalInput")
    wu = nc.dram_tensor("wu", [MC, P, KD, P], bf16, kind="ExternalInput")
    wo = nc.dram_tensor("wo", [DC, P, KM, P], bf16, kind="ExternalInput")
    wrep = nc.dram_tensor("wrep", [P, C], f32, kind="ExternalInput")
    out = nc.dram_tensor("out", [DC, P, C], f32, kind="ExternalOutput")

    tch = [(i * TCHUNK, min(TCHUNK, C - i * TCHUNK)) for i in range((C + TCHUNK - 1) // TCHUNK)]

    with tile.TileContext(nc) as tc:
        with (
            tc.For_i(0, R, 1) if R > 1 else contextlib.nullcontext(),
            tc.tile_pool(name="big", bufs=1) as big,
            tc.tile_pool(name="wpool", bufs=4) as wpool,
            tc.tile_pool(name="wopool", bufs=3) as wopool,
            tc.tile_pool(name="tmp", bufs=3) as tmp,
            tc.tile_pool(name="psg_pool", bufs=4, space="PSUM") as psg_pool,
            tc.tile_pool(name="psu_pool", bufs=4, space="PSUM") as psu_pool,
        ):
            # xt split per k-subtile so the first matmuls start after 1/8 of
            # the activation DMA instead of all of it.
            xt_sb = big.tile([P, KD, C], bf16)
            for k in range(KD):
                nc.sync.dma_start(xt_sb[:, k, :], xt[:, k, :])
            wrep_sb = big.tile([P, C], f32)
            nc.sync.dma_start(wrep_sb[:], wrep[:])
            # Prefetch the first phase-B weight chunk during phase A.
            wo_sb0 = wopool.tile([P, KM, P], bf16, tag="wo")
            nc.sync.dma_start(wo_sb0[:], wo[0])
            h_sb = big.tile([P, KM, C], bf16)

            # ---- phase A: hT[m, t] = silu(gateT) * upT over 16 m-chunks ----
            # k-outer / t-inner: consecutive matmuls share the stationary
            # weight chunk (the redundant LDWEIGHTS pipeline much better).
            for mc in range(MC):
                wg_sb = wpool.tile([P, KD, P], bf16, tag="wg")
                nc.sync.dma_start(wg_sb[:], wg[mc])
                wu_sb = wpool.tile([P, KD, P], bf16, tag="wu")
                nc.sync.dma_start(wu_sb[:], wu[mc])
                ps_gs = [psg_pool.tile([P, TCHUNK], f32, tag="psg", name=f"psg{i}")
                         for i in range(len(tch))]
                ps_us = [psu_pool.tile([P, TCHUNK], f32, tag="psu", name=f"psu{i}")
                         for i in range(len(tch))]
                for k in range(KD):
                    for i, (t0, tw) in enumerate(tch):
                        nc.tensor.matmul(
                            ps_gs[i][:, :tw], wg_sb[:, k, :],
                            xt_sb[:, k, t0 : t0 + tw],
                            start=(k == 0), stop=(k == KD - 1),
                        )
                for k in range(KD):
                    for i, (t0, tw) in enumerate(tch):
                        nc.tensor.matmul(
                            ps_us[i][:, :tw], wu_sb[:, k, :],
                            xt_sb[:, k, t0 : t0 + tw],
                            start=(k == 0), stop=(k == KD - 1),
                        )
                for i, (t0, tw) in enumerate(tch):
                    g_sb = tmp.tile([P, TCHUNK], bf16, tag="g")
                    nc.scalar.activation(
                        g_sb[:, :tw], ps_gs[i][:, :tw],
                        func=mybir.ActivationFunctionType.Silu,
                    )
                    nc.vector.tensor_mul(
                        h_sb[:, mc, t0 : t0 + tw], g_sb[:, :tw], ps_us[i][:, :tw]
                    )

            # ---- phase B: yT[d, t] = (hT.T @ Wo).T * w[t] over 8 d-chunks ----
            # psum tiles reuse the phase-A "psg" slots (phases are sequential)
            for dc in range(DC):
                if dc == 0:
                    wo_sb = wo_sb0
                else:
                    wo_sb = wopool.tile([P, KM, P], bf16, tag="wo")
                    nc.sync.dma_start(wo_sb[:], wo[dc])
                ps_ys = [psg_pool.tile([P, TCHUNK], f32, tag="psg", name=f"psy{i}")
                         for i in range(len(tch))]
                for k in range(KM):
                    for i, (t0, tw) in enumerate(tch):
                        nc.tensor.matmul(
                            ps_ys[i][:, :tw], wo_sb[:, k, :],
                            h_sb[:, k, t0 : t0 + tw],
                            start=(k == 0), stop=(k == KM - 1),
                        )
                for i, (t0, tw) in enumerate(tch):
                    o_sb = tmp.tile([P, TCHUNK], f32, tag="o")
                    nc.vector.tensor_mul(
                        o_sb[:, :tw], ps_ys[i][:, :tw], wrep_sb[:, t0 : t0 + tw]
                    )
                    nc.sync.dma_start(out[dc, :, t0 : t0 + tw], o_sb[:, :tw])

    nc.compile()
    return nc


class _Runner:
    """Persistent jitted SPMD executor (mirrors bass2jax.run_bass_via_pjrt,
    but reusable across calls so repeated runs skip retrace/recompile)."""

    def __init__(self, nc, n_cores=8):
        import jax
        from jax.sharding import Mesh, PartitionSpec
        from jax.experimental.shard_map import shard_map
        import concourse.mybir as mybir
        from concourse import bass2jax

        bass2jax.install_neuronx_cc_hook()
        self.jax = jax
        self.n_cores = n_cores

        partition_name = (
            nc.partition_id_tensor.name if nc.partition_id_tensor else None
        )
        in_names, out_names, out_avals, zero_outs = [], [], [], []
        for alloc in nc.m.functions[0].allocations:
            if not isinstance(alloc, mybir.MemoryLocationSet):
                continue
            name = alloc.memorylocations[0].name
            if alloc.kind == "ExternalInput":
                if name != partition_name:
                    in_names.append(name)
            elif alloc.kind == "ExternalOutput":
                shape = tuple(alloc.tensor_shape)
                dtype = mybir.dt.np(alloc.dtype)
                out_names.append(name)
                out_avals.append(jax.core.ShapedArray(shape, dtype))
                zero_outs.append(np.zeros(shape, dtype))
        self.in_names = list(in_names)
        self.out_names = list(out_names)
        self.out_avals = out_avals
        n_params = len(in_names)
        all_in_names = in_names + out_names
        if partition_name is not None:
            all_in_names = all_in_names + [partition_name]

        def _call_once(operands):
            return bass2jax._bass_exec_p.bind(
                *operands,
                out_avals=tuple(out_avals),
                in_names=tuple(all_in_names),
                out_names=tuple(out_names),
                lowering_input_output_aliases=(),
                sim_require_finite=True,
                sim_require_nnan=True,
                nc=nc,
            )

        def _make_body(reps):
            def _body(*args):
                operands = list(args)
                if partition_name is not None:
                    operands.append(bass2jax.partition_id_tensor())
                outs = _call_once(operands)
                for _ in range(reps - 1):
                    outs = _call_once(operands)
                return tuple(outs)

            return _body

        devices = jax.devices()[:n_cores]
        assert len(devices) == n_cores
        mesh = Mesh(np.asarray(devices), ("core",))
        in_specs = (PartitionSpec("core"),) * (n_params + len(out_names))
        out_specs = (PartitionSpec("core"),) * len(out_names)

        def _jit(reps):
            return jax.jit(
                shard_map(_make_body(reps), mesh=mesh, in_specs=in_specs,
                          out_specs=out_specs, check_rep=False),
                keep_unused=True,
            )

        self._fns = {}
        self._jit = _jit
        self._fn = self.get_fn(1)
        self._zero_concat = [
            np.zeros((n_cores * z.shape[0], *z.shape[1:]), z.dtype) for z in zero_outs
        ]

    def run(self, in_maps):
        concat_in = [
            np.concatenate([np.asarray(m[name]) for m in in_maps], axis=0)
            for name in self.in_names
        ]
        t0 = time.time()
        out_arrs = self._fn(*concat_in, *self._zero_concat)
        out_arrs = [np.asarray(a) for a in out_arrs]
        LAST_RUN["run_s"] = time.time() - t0
        return [
            {
                name: out_arrs[i].reshape(self.n_cores, *self.out_avals[i].shape)[c]
                for i, name in enumerate(self.out_names)
            }
            for c in range(self.n_cores)
        ]

    def get_fn(self, reps):
        if reps not in self._fns:
            self._fns[reps] = self._jit(reps)
        return self._fns[reps]

    def _time_fn(self, fn, dev_in, dev_zero, iters):
        jax = self.jax
        r = fn(*dev_in, *dev_zero)  # warmup / compile
        jax.block_until_ready(r)
        times = []
        for _ in range(iters):
            t0 = time.perf_counter()
            r = fn(*dev_in, *dev_zero)
            jax.block_until_ready(r)
            times.append(time.perf_counter() - t0)
        return min(times)

    def bench(self, in_maps, iters=3, reps=8):
        """Time reps-in-one-launch vs 1; slope isolates per-NEFF-exec time
        from axon dispatch overhead."""
        concat_in = [
            np.concatenate([np.asarray(m[name]) for m in in_maps], axis=0)
            for name in self.in_names
        ]
        jax = self.jax
        dev_in = [jax.device_put(a) for a in concat_in]
        dev_zero = [jax.device_put(a) for a in self._zero_concat]
        t1 = self._time_fn(self.get_fn(1), dev_in, dev_zero, iters)
        tn = self._time_fn(self.get_fn(reps), dev_in, dev_zero, iters)
        per_exec = (tn - t1) / (reps - 1)
        return {"t1_s": t1, "tn_s": tn, "reps": reps, "per_exec_s": per_exec}


def _route(residual: np.ndarray, W_router: np.ndarray):
    """Host router: softmax over experts, top-2 (desc, ties -> lower idx),
    renormalize. Returns per-expert (token_ids, weights)."""
    X = residual.reshape(N, D).astype(np.float32)
    logits = X @ W_router.astype(np.float32)
    mx = logits.max(axis=-1, keepdims=True)
    e = np.exp(logits - mx)
    probs = e / e.sum(axis=-1, keepdims=True)
    order = np.argsort(-probs, axis=-1, kind="stable")[:, :TOP_K]       # [N, 2]
    vals = np.take_along_axis(probs, order, axis=-1)                     # [N, 2]
    wts = vals / (vals.sum(axis=-1, keepdims=True) + 1e-8)
    ids, ws = [], []
    for ex in range(E):
        hit = order == ex                                                # [N, 2]
        sel = np.nonzero(hit.any(axis=-1))[0]
        w_tok = np.where(hit[sel, 0], wts[sel, 0], wts[sel, 1]).astype(np.float32)
        ids.append(sel)
        ws.append(w_tok)
    return X, ids, ws


def kernel(
    residual, W_router, W_gate, b_gate, W_up, b_up, W_out, b_out
) -> np.ndarray:
    # NOTE: b_gate/b_up/b_out have fill=zeros in the problem spec and are
    # therefore not applied on-device.
    import ml_dtypes

    bf16 = ml_dtypes.bfloat16

    t_host0 = time.time()
    X, ids, ws = _route(np.asarray(residual), np.asarray(W_router))
    counts = [len(s) for s in ids]
    C = max(TCHUNK, ((max(counts) + 63) // 64) * 64)

    X16 = X.astype(bf16)
    W_gate = np.asarray(W_gate, dtype=np.float32).astype(bf16)
    W_up = np.asarray(W_up, dtype=np.float32).astype(bf16)
    W_out = np.asarray(W_out, dtype=np.float32).astype(bf16)

    in_maps = []
    for ex in range(E):
        n_e = counts[ex]
        xt = np.zeros((P, KD, C), bf16)
        xt[:, :, :n_e] = X16[ids[ex]].T.reshape(KD, P, n_e).transpose(1, 0, 2)
        wrep = np.zeros((P, C), np.float32)
        wrep[:, :n_e] = ws[ex][None, :]
        in_maps.append(
            {
                "xt": xt,
                "wg": np.ascontiguousarray(
                    W_gate[ex].reshape(KD, P, MC, P).transpose(2, 1, 0, 3)
                ),
                "wu": np.ascontiguousarray(
                    W_up[ex].reshape(KD, P, MC, P).transpose(2, 1, 0, 3)
                ),
                "wo": np.ascontiguousarray(
                    W_out[ex].reshape(KM, P, DC, P).transpose(2, 1, 0, 3)
                ),
                "wrep": wrep,
            }
        )
    LAST_RUN["host_prep_s"] = time.time() - t_host0
    LAST_RUN["C"] = C
    LAST_RUN["counts"] = counts
    LAST_RUN["in_maps"] = in_maps

    if C not in _runner_cache:
        t0 = time.time()
        nc = _build_bass(C)
        LAST_RUN["build_s"] = time.time() - t0
        _runner_cache[C] = _Runner(nc)
    runner = _runner_cache[C]
    results = runner.run(in_maps)

    res = np.zeros((N, D), np.float32)
    for ex in range(E):
        n_e = counts[ex]
        y = results[ex]["out"].reshape(D, C)[:, :n_e]                    # [D, n_e]
        res[ids[ex]] += y.T
    return res.reshape(B, S, D)


def get_runner(C: int):
    return _runner_cache.get(C)


# revision 6
# speedup vs baseline: 1.4456x; 1.2686x over previous
"""MoE SwiGLU MLP (top-2 of 8 experts) on 8 Trainium2 NeuronCores.

Strategy: expert-parallel with token routing. The router (a 1024x8 matmul +
softmax + top-2) is tiny, so it runs on the host as part of sharding. Each
core is assigned one expert and receives only the tokens routed to it
(gathered + transposed on the host into PE-friendly layouts, converted to
bf16). On-device each core runs a dense SwiGLU MLP over its [C, 1024] token
slab with bf16 matmuls (f32 PSUM accumulation), scales by the renormalized
router weight, and the host scatter-adds the two per-token expert
contributions back into the full [2, 2048, 1024] output.

bf16 matters twice: the PE streams 1 column/cycle at any chunk width (f32r
drops to 1/4 rate below 256 columns, which made the old 128-wide tail chunk
cost 4x), and weight DMA bytes halve so the per-expert 12.6MB streams well
under the compute time.
"""

import time

import numpy as np

B, S, D, M, E, TOP_K = 2, 2048, 1024, 2048, 8, 2
N = B * S
P = 128
KD = D // P   # 8  k-subtiles over the d contraction
KM = M // P   # 16 k-subtiles over the m contraction
MC = M // P   # 16 m-chunks (phase A output partitions)
DC = D // P   # 8  d-chunks (phase B output partitions)
TCHUNK = 512

_runner_cache: dict[int, object] = {}
LAST_RUN: dict = {}


def _build_bass(C: int, R: int = 1):
    import contextlib

    import concourse.bacc as bacc
    import concourse.mybir as mybir
    import concourse.tile as tile

    f32 = mybir.dt.float32
    bf16 = mybir.dt.bfloat16

    nc = bacc.Bacc("TRN2", target_bir_lowering=False, debug=False, num_devices=8)

    xt = nc.dram_tensor("xt", [P, KD, C], bf16, kind="ExternalInput")
    wg = nc.dram_tensor("wg", [MC, P, KD, P], bf16, kind="ExternalInput")
    wu = nc.dram_tensor("wu", [MC, P, KD, P], bf16, kind="ExternalInput")
    wo = nc.dram_tensor("wo", [DC, P, KM, P], bf16, kind="ExternalInput")
    wrep = nc.dram_tensor("wrep", [P, C], f32, kind="ExternalInput")
    out = nc.dram_tensor("out", [DC, P, C], f32, kind="ExternalOutput")

    tch = [(i * TCHUNK, min(TCHUNK, C - i * TCHUNK)) for i in range((C + TCHUNK - 1) // TCHUNK)]

    with tile.TileContext(nc) as tc:
        with (
            tc.For_i(0, R, 1) if R > 1 else contextlib.nullcontext(),
            tc.tile_pool(name="big", bufs=1) as big,
            tc.tile_pool(name="wpool", bufs=4) as wpool,
            tc.tile_pool(name="wopool", bufs=3) as wopool,
            tc.tile_pool(name="tmp", bufs=3) as tmp,
            tc.tile_pool(name="psg_pool", bufs=4, space="PSUM") as psg_pool,
            tc.tile_pool(name="psu_pool", bufs=4, space="PSUM") as psu_pool,
        ):
            # DMA order matters: the packets round-robin across queues in
            # program order, so everything emitted before the first matmul's
            # dependencies delays it. Emit wg0/wu0 + xt[k0] first; defer
            # wrep / the wo prefetch into the mc loop.
            xt_sb = big.tile([P, KD, C], bf16)
            wrep_sb = big.tile([P, C], f32)
            wo_sb0 = wopool.tile([P, KM, P], bf16, tag="wo")
            h_sb = big.tile([P, KM, C], bf16)

            # ---- phase A: hT[m, t] = silu(gateT) * upT over 16 m-chunks ----
            # k-outer / t-inner: consecutive matmuls share the stationary
            # weight chunk (the redundant LDWEIGHTS pipeline much better).
            for mc in range(MC):
                wg_sb = wpool.tile([P, KD, P], bf16, tag="wg")
                nc.sync.dma_start(wg_sb[:], wg[mc])
                wu_sb = wpool.tile([P, KD, P], bf16, tag="wu")
                nc.sync.dma_start(wu_sb[:], wu[mc])
                if mc == 0:
                    for k in range(KD):
                        nc.sync.dma_start(xt_sb[:, k, :], xt[:, k, :])
                elif mc == 1:
                    nc.sync.dma_start(wrep_sb[:], wrep[:])
                    nc.sync.dma_start(wo_sb0[:], wo[0])
                ps_gs = [psg_pool.tile([P, TCHUNK], f32, tag="psg", name=f"psg{i}")
                         for i in range(len(tch))]
                ps_us = [psu_pool.tile([P, TCHUNK], f32, tag="psu", name=f"psu{i}")
                         for i in range(len(tch))]
                for k in range(KD):
                    for i, (t0, tw) in enumerate(tch):
                        nc.tensor.matmul(
                            ps_gs[i][:, :tw], wg_sb[:, k, :],
                            xt_sb[:, k, t0 : t0 + tw],
                            start=(k == 0), stop=(k == KD - 1),
                        )
                for k in range(KD):
                    for i, (t0, tw) in enumerate(tch):
                        nc.tensor.matmul(
                            ps_us[i][:, :tw], wu_sb[:, k, :],
                            xt_sb[:, k, t0 : t0 + tw],
                            start=(k == 0), stop=(k == KD - 1),
                        )
                for i, (t0, tw) in enumerate(tch):
                    g_sb = tmp.tile([P, TCHUNK], bf16, tag="g")
                    nc.scalar.activation(
                        g_sb[:, :tw], ps_gs[i][:, :tw],
                        func=mybir.ActivationFunctionType.Silu,
                    )
                    nc.vector.tensor_mul(
                        h_sb[:, mc, t0 : t0 + tw], g_sb[:, :tw], ps_us[i][:, :tw]
                    )

            # ---- phase B: yT[d, t] = (hT.T @ Wo).T * w[t] over 8 d-chunks ----
            # psum tiles reuse the phase-A "psg" slots (phases are sequential)
            for dc in range(DC):
                if dc == 0:
                    wo_sb = wo_sb0
                else:
                    wo_sb = wopool.tile([P, KM, P], bf16, tag="wo")
                    nc.sync.dma_start(wo_sb[:], wo[dc])
                ps_ys = [psg_pool.tile([P, TCHUNK], f32, tag="psg", name=f"psy{i}")
                         for i in range(len(tch))]
                for k in range(KM):
                    for i, (t0, tw) in enumerate(tch):
                        nc.tensor.matmul(
                            ps_ys[i][:, :tw], wo_sb[:, k, :],
                            h_sb[:, k, t0 : t0 + tw],
                            start=(k == 0), stop=(k == KM - 1),
                        )
                for i, (t0, tw) in enumerate(tch):
                    o_sb = tmp.tile([P, TCHUNK], f32, tag="o")
                    nc.vector.tensor_mul(
                        o_sb[:, :tw], ps_ys[i][:, :tw], wrep_sb[:, t0 : t0 + tw]
                    )
                    nc.sync.dma_start(out[dc, :, t0 : t0 + tw], o_sb[:, :tw])

    nc.compile()
    return nc


class _Runner:
    """Persistent jitted SPMD executor (mirrors bass2jax.run_bass_via_pjrt,
    but reusable across calls so repeated runs skip retrace/recompile)."""

    def __init__(self, nc, n_cores=8):
        import jax
        from jax.sharding import Mesh, PartitionSpec
        from jax.experimental.shard_map import shard_map
        import concourse.mybir as mybir
        from concourse import bass2jax

        bass2jax.install_neuronx_cc_hook()
        self.jax = jax
        self.n_cores = n_cores

        partition_name = (
            nc.partition_id_tensor.name if nc.partition_id_tensor else None
        )
        in_names, out_names, out_avals, zero_outs = [], [], [], []
        for alloc in nc.m.functions[0].allocations:
            if not isinstance(alloc, mybir.MemoryLocationSet):
                continue
            name = alloc.memorylocations[0].name
            if alloc.kind == "ExternalInput":
                if name != partition_name:
                    in_names.append(name)
            elif alloc.kind == "ExternalOutput":
                shape = tuple(alloc.tensor_shape)
                dtype = mybir.dt.np(alloc.dtype)
                out_names.append(name)
                out_avals.append(jax.core.ShapedArray(shape, dtype))
                zero_outs.append(np.zeros(shape, dtype))
        self.in_names = list(in_names)
        self.out_names = list(out_names)
        self.out_avals = out_avals
        n_params = len(in_names)
        all_in_names = in_names + out_names
        if partition_name is not None:
            all_in_names = all_in_names + [partition_name]

        def _call_once(operands):
            return bass2jax._bass_exec_p.bind(
                *operands,
                out_avals=tuple(out_avals),
                in_names=tuple(all_in_names),
                out_names=tuple(out_names),
                lowering_input_output_aliases=(),
                sim_require_finite=True,
                sim_require_nnan=True,
                nc=nc,
            )

        def _make_body(reps):
            def _body(*args):
                operands = list(args)
                if partition_name is not None:
                    operands.append(bass2jax.partition_id_tensor())
                outs = _call_once(operands)
                for _ in range(reps - 1):
                    outs = _call_once(operands)
                return tuple(outs)

            return _body

        devices = jax.devices()[:n_cores]
        assert len(devices) == n_cores
        mesh = Mesh(np.asarray(devices), ("core",))
        in_specs = (PartitionSpec("core"),) * (n_params + len(out_names))
        out_specs = (PartitionSpec("core"),) * len(out_names)

        def _jit(reps):
            return jax.jit(
                shard_map(_make_body(reps), mesh=mesh, in_specs=in_specs,
                          out_specs=out_specs, check_rep=False),
                keep_unused=True,
            )

        self._fns = {}
        self._jit = _jit
        self._fn = self.get_fn(1)
        self._zero_concat = [
            np.zeros((n_cores * z.shape[0], *z.shape[1:]), z.dtype) for z in zero_outs
        ]

    def run(self, in_maps):
        concat_in = [
            np.concatenate([np.asarray(m[name]) for m in in_maps], axis=0)
            for name in self.in_names
        ]
        t0 = time.time()
        out_arrs = self._fn(*concat_in, *self._zero_concat)
        out_arrs = [np.asarray(a) for a in out_arrs]
        LAST_RUN["run_s"] = time.time() - t0
        return [
            {
                name: out_arrs[i].reshape(self.n_cores, *self.out_avals[i].shape)[c]
                for i, name in enumerate(self.out_names)
            }
            for c in range(self.n_cores)
        ]

    def get_fn(self, reps):
        if reps not in self._fns:
            self._fns[reps] = self._jit(reps)
        return self._fns[reps]

    def _time_fn(self, fn, dev_in, dev_zero, iters):
        jax = self.jax
        r = fn(*dev_in, *dev_zero)  # warmup / compile
        jax.block_until_ready(r)
        times = []
        for _ in range(iters):
            t0 = time.perf_counter()
            r = fn(*dev_in, *dev_zero)
            jax.block_until_ready(r)
            times.append(time.perf_counter() - t0)
        return min(times)

    def bench(self, in_maps, iters=3, reps=8):
        """Time reps-in-one-launch vs 1; slope isolates per-NEFF-exec time
        from axon dispatch overhead."""
        concat_in = [
            np.concatenate([np.asarray(m[name]) for m in in_maps], axis=0)
            for name in self.in_names
        ]
        jax = self.jax
        dev_in = [jax.device_put(a) for a in concat_in]
        dev_zero = [jax.device_put(a) for a in self._zero_concat]
        t1 = self._time_fn(self.get_fn(1), dev_in, dev_zero, iters)
        tn = self._time_fn(self.get_fn(reps), dev_in, dev_zero, iters)
        per_exec = (tn - t1) / (reps - 1)
        return {"t1_s": t1, "tn_s": tn, "reps": reps, "per_exec_s": per_exec}


def _route(residual: np.ndarray, W_router: np.ndarray):
    """Host router: softmax over experts, top-2 (desc, ties -> lower idx),
    renormalize. Returns per-expert (token_ids, weights)."""
    X = residual.reshape(N, D).astype(np.float32)
    logits = X @ W_router.astype(np.float32)
    mx = logits.max(axis=-1, keepdims=True)
    e = np.exp(logits - mx)
    probs = e / e.sum(axis=-1, keepdims=True)
    order = np.argsort(-probs, axis=-1, kind="stable")[:, :TOP_K]       # [N, 2]
    vals = np.take_along_axis(probs, order, axis=-1)                     # [N, 2]
    wts = vals / (vals.sum(axis=-1, keepdims=True) + 1e-8)
    ids, ws = [], []
    for ex in range(E):
        hit = order == ex                                                # [N, 2]
        sel = np.nonzero(hit.any(axis=-1))[0]
        w_tok = np.where(hit[sel, 0], wts[sel, 0], wts[sel, 1]).astype(np.float32)
        ids.append(sel)
        ws.append(w_tok)
    return X, ids, ws


def kernel(
    residual, W_router, W_gate, b_gate, W_up, b_up, W_out, b_out
) -> np.ndarray:
    # NOTE: b_gate/b_up/b_out have fill=zeros in the problem spec and are
    # therefore not applied on-device.
    import ml_dtypes

    bf16 = ml_dtypes.bfloat16

    t_host0 = time.time()
    X, ids, ws = _route(np.asarray(residual), np.asarray(W_router))
    counts = [len(s) for s in ids]
    C = max(TCHUNK, ((max(counts) + 31) // 32) * 32)

    X16 = X.astype(bf16)
    W_gate = np.asarray(W_gate, dtype=np.float32).astype(bf16)
    W_up = np.asarray(W_up, dtype=np.float32).astype(bf16)
    W_out = np.asarray(W_out, dtype=np.float32).astype(bf16)

    in_maps = []
    for ex in range(E):
        n_e = counts[ex]
        xt = np.zeros((P, KD, C), bf16)
        xt[:, :, :n_e] = X16[ids[ex]].T.reshape(KD, P, n_e).transpose(1, 0, 2)
        wrep = np.zeros((P, C), np.float32)
        wrep[:, :n_e] = ws[ex][None, :]
        in_maps.append(
            {
                "xt": xt,
                "wg": np.ascontiguousarray(
                    W_gate[ex].reshape(KD, P, MC, P).transpose(2, 1, 0, 3)
                ),
                "wu": np.ascontiguousarray(
                    W_up[ex].reshape(KD, P, MC, P).transpose(2, 1, 0, 3)
                ),
                "wo": np.ascontiguousarray(
                    W_out[ex].reshape(KM, P, DC, P).transpose(2, 1, 0, 3)
                ),
                "wrep": wrep,
            }
        )
    LAST_RUN["host_prep_s"] = time.time() - t_host0
    LAST_RUN["C"] = C
    LAST_RUN["counts"] = counts
    LAST_RUN["in_maps"] = in_maps

    if C not in _runner_cache:
        t0 = time.time()
        nc = _build_bass(C)
        LAST_RUN["build_s"] = time.time() - t0
        _runner_cache[C] = _Runner(nc)
    runner = _runner_cache[C]
    results = runner.run(in_maps)

    res = np.zeros((N, D), np.float32)
    for ex in range(E):
        n_e = counts[ex]
        y = results[ex]["out"].reshape(D, C)[:, :n_e]                    # [D, n_e]
        res[ids[ex]] += y.T
    return res.reshape(B, S, D)


def get_runner(C: int):
    return _runner_cache.get(C)


# revision 7
# speedup vs baseline: 1.4650x; 1.0134x over previous
"""MoE SwiGLU MLP (top-2 of 8 experts) on 8 Trainium2 NeuronCores.

Strategy: expert-parallel with token routing. The router (a 1024x8 matmul +
softmax + top-2) is tiny, so it runs on the host as part of sharding. Each
core runs a dense SwiGLU MLP over a [C, 1024] token slab with bf16 matmuls
(f32 PSUM accumulation), scales by the renormalized router weight, and the
host scatter-adds the per-token expert contributions back into the full
[2, 2048, 1024] output.

Load balancing without per-core control flow: the PE is fill-rate-bound
(1 column/cycle), so per-core streamed columns C set the time. Columns
[0, XCOLS) hold the core's primary expert (first XCOLS routed tokens);
columns [XCOLS, C) are a small spill chunk holding overflow tokens of some
(possibly different) expert, with its own full weight set (wg2/wu2/wo2).
All cores run the identical program; only the input tensors differ. Unused
spill slots get zero weights/tokens and contribute zeros.
"""

import time

import numpy as np

B, S, D, M, E, TOP_K = 2, 2048, 1024, 2048, 8, 2
N = B * S
P = 128
KD = D // P   # 8  k-subtiles over the d contraction
KM = M // P   # 16 k-subtiles over the m contraction
MC = M // P   # 16 m-chunks (phase A output partitions)
DC = D // P   # 8  d-chunks (phase B output partitions)
TCHUNK = 512
XCOLS = 1024  # primary-expert column budget per core

_runner_cache: dict = {}
LAST_RUN: dict = {}


def _build_bass(C: int, R: int = 1):
    """C = XCOLS + Y (spill kernel, Y>0) or C <= XCOLS (no spill)."""
    import contextlib

    import concourse.bacc as bacc
    import concourse.mybir as mybir
    import concourse.tile as tile

    f32 = mybir.dt.float32
    bf16 = mybir.dt.bfloat16

    Y = C - XCOLS if C > XCOLS else 0

    nc = bacc.Bacc("TRN2", target_bir_lowering=False, debug=False, num_devices=8)

    xt = nc.dram_tensor("xt", [P, KD, C], bf16, kind="ExternalInput")
    wg = nc.dram_tensor("wg", [MC, P, KD, P], bf16, kind="ExternalInput")
    wu = nc.dram_tensor("wu", [MC, P, KD, P], bf16, kind="ExternalInput")
    wo = nc.dram_tensor("wo", [DC, P, KM, P], bf16, kind="ExternalInput")
    if Y:
        wg2 = nc.dram_tensor("wg2", [MC, P, KD, P], bf16, kind="ExternalInput")
        wu2 = nc.dram_tensor("wu2", [MC, P, KD, P], bf16, kind="ExternalInput")
        wo2 = nc.dram_tensor("wo2", [DC, P, KM, P], bf16, kind="ExternalInput")
    wrep = nc.dram_tensor("wrep", [P, C], f32, kind="ExternalInput")
    out = nc.dram_tensor("out", [DC, P, C], f32, kind="ExternalOutput")

    # X chunks of <=512 plus the Y spill chunk; each chunk knows its weight set
    xc = XCOLS if Y else C
    tch = [(i * TCHUNK, min(TCHUNK, xc - i * TCHUNK), 0)
           for i in range((xc + TCHUNK - 1) // TCHUNK)]
    if Y:
        tch.append((XCOLS, Y, 1))

    with tile.TileContext(nc) as tc:
        with (
            tc.For_i(0, R, 1) if R > 1 else contextlib.nullcontext(),
            tc.tile_pool(name="big", bufs=1) as big,
            tc.tile_pool(name="wpool", bufs=4) as wpool,
            tc.tile_pool(name="wopool", bufs=3) as wopool,
            tc.tile_pool(name="tmp", bufs=3) as tmp,
            tc.tile_pool(name="psg_pool", bufs=4, space="PSUM") as psg_pool,
            tc.tile_pool(name="psu_pool", bufs=4, space="PSUM") as psu_pool,
        ):
            # DMA order matters: packets round-robin across queues in program
            # order, so anything emitted before the first matmul's inputs
            # delays it. wg0/wu0 + xt go first; the rest trickles in inside
            # the mc loop.
            xt_sb = big.tile([P, KD, C], bf16)
            wrep_sb = big.tile([P, C], f32)
            wo_sb0 = wopool.tile([P, KM, P], bf16, tag="wo")
            if Y:
                wo2_sb0 = wopool.tile([P, KM, P], bf16, tag="wo2")
            h_sb = big.tile([P, KM, C], bf16)

            # ---- phase A: hT[m, t] = silu(gateT) * upT over 16 m-chunks ----
            for mc in range(MC):
                wg_sb = wpool.tile([P, KD, P], bf16, tag="wg")
                nc.sync.dma_start(wg_sb[:], wg[mc])
                wu_sb = wpool.tile([P, KD, P], bf16, tag="wu")
                nc.sync.dma_start(wu_sb[:], wu[mc])
                if Y:
                    wg2_sb = wpool.tile([P, KD, P], bf16, tag="wg2")
                    nc.sync.dma_start(wg2_sb[:], wg2[mc])
                    wu2_sb = wpool.tile([P, KD, P], bf16, tag="wu2")
                    nc.sync.dma_start(wu2_sb[:], wu2[mc])
                if mc == 0:
                    for k in range(KD):
                        nc.sync.dma_start(xt_sb[:, k, :], xt[:, k, :])
                elif mc == 1:
                    nc.sync.dma_start(wrep_sb[:], wrep[:])
                    nc.sync.dma_start(wo_sb0[:], wo[0])
                    if Y:
                        nc.sync.dma_start(wo2_sb0[:], wo2[0])
                ps_gs = [psg_pool.tile([P, TCHUNK], f32, tag="psg", name=f"psg{i}")
                         for i in range(len(tch))]
                ps_us = [psu_pool.tile([P, TCHUNK], f32, tag="psu", name=f"psu{i}")
                         for i in range(len(tch))]
                for k in range(KD):
                    for i, (t0, tw, ws) in enumerate(tch):
                        w_sb = wg2_sb if ws else wg_sb
                        nc.tensor.matmul(
                            ps_gs[i][:, :tw], w_sb[:, k, :],
                            xt_sb[:, k, t0 : t0 + tw],
                            start=(k == 0), stop=(k == KD - 1),
                        )
                for k in range(KD):
                    for i, (t0, tw, ws) in enumerate(tch):
                        w_sb = wu2_sb if ws else wu_sb
                        nc.tensor.matmul(
                            ps_us[i][:, :tw], w_sb[:, k, :],
                            xt_sb[:, k, t0 : t0 + tw],
                            start=(k == 0), stop=(k == KD - 1),
                        )
                for i, (t0, tw, ws) in enumerate(tch):
                    g_sb = tmp.tile([P, TCHUNK], bf16, tag="g")
                    nc.scalar.activation(
                        g_sb[:, :tw], ps_gs[i][:, :tw],
                        func=mybir.ActivationFunctionType.Silu,
                    )
                    nc.vector.tensor_mul(
                        h_sb[:, mc, t0 : t0 + tw], g_sb[:, :tw], ps_us[i][:, :tw]
                    )

            # ---- phase B: yT[d, t] = (hT.T @ Wo).T * w[t] over 8 d-chunks ----
            for dc in range(DC):
                if dc == 0:
                    wo_sb = wo_sb0
                    wo2_sb = wo2_sb0 if Y else None
                else:
                    wo_sb = wopool.tile([P, KM, P], bf16, tag="wo")
                    nc.sync.dma_start(wo_sb[:], wo[dc])
                    if Y:
                        wo2_sb = wopool.tile([P, KM, P], bf16, tag="wo2")
                        nc.sync.dma_start(wo2_sb[:], wo2[dc])
                ps_ys = [psg_pool.tile([P, TCHUNK], f32, tag="psg", name=f"psy{i}")
                         for i in range(len(tch))]
                for k in range(KM):
                    for i, (t0, tw, ws) in enumerate(tch):
                        w_sb = wo2_sb if ws else wo_sb
                        nc.tensor.matmul(
                            ps_ys[i][:, :tw], w_sb[:, k, :],
                            h_sb[:, k, t0 : t0 + tw],
                            start=(k == 0), stop=(k == KM - 1),
                        )
                for i, (t0, tw, ws) in enumerate(tch):
                    o_sb = tmp.tile([P, TCHUNK], f32, tag="o")
                    nc.vector.tensor_mul(
                        o_sb[:, :tw], ps_ys[i][:, :tw], wrep_sb[:, t0 : t0 + tw]
                    )
                    nc.sync.dma_start(out[dc, :, t0 : t0 + tw], o_sb[:, :tw])

    nc.compile()
    return nc


class _Runner:
    """Persistent jitted SPMD executor (mirrors bass2jax.run_bass_via_pjrt,
    but reusable across calls so repeated runs skip retrace/recompile)."""

    def __init__(self, nc, n_cores=8):
        import jax
        from jax.sharding import Mesh, PartitionSpec
        from jax.experimental.shard_map import shard_map
        import concourse.mybir as mybir
        from concourse import bass2jax

        bass2jax.install_neuronx_cc_hook()
        self.jax = jax
        self.n_cores = n_cores

        partition_name = (
            nc.partition_id_tensor.name if nc.partition_id_tensor else None
        )
        in_names, out_names, out_avals, zero_outs = [], [], [], []
        for alloc in nc.m.functions[0].allocations:
            if not isinstance(alloc, mybir.MemoryLocationSet):
                continue
            name = alloc.memorylocations[0].name
            if alloc.kind == "ExternalInput":
                if name != partition_name:
                    in_names.append(name)
            elif alloc.kind == "ExternalOutput":
                shape = tuple(alloc.tensor_shape)
                dtype = mybir.dt.np(alloc.dtype)
                out_names.append(name)
                out_avals.append(jax.core.ShapedArray(shape, dtype))
                zero_outs.append(np.zeros(shape, dtype))
        self.in_names = list(in_names)
        self.out_names = list(out_names)
        self.out_avals = out_avals
        n_params = len(in_names)
        all_in_names = in_names + out_names
        if partition_name is not None:
            all_in_names = all_in_names + [partition_name]

        def _call_once(operands):
            return bass2jax._bass_exec_p.bind(
                *operands,
                out_avals=tuple(out_avals),
                in_names=tuple(all_in_names),
                out_names=tuple(out_names),
                lowering_input_output_aliases=(),
                sim_require_finite=True,
                sim_require_nnan=True,
                nc=nc,
            )

        def _make_body(reps):
            def _body(*args):
                operands = list(args)
                if partition_name is not None:
                    operands.append(bass2jax.partition_id_tensor())
                outs = _call_once(operands)
                for _ in range(reps - 1):
                    outs = _call_once(operands)
                return tuple(outs)

            return _body

        devices = jax.devices()[:n_cores]
        assert len(devices) == n_cores
        mesh = Mesh(np.asarray(devices), ("core",))
        in_specs = (PartitionSpec("core"),) * (n_params + len(out_names))
        out_specs = (PartitionSpec("core"),) * len(out_names)

        def _jit(reps):
            return jax.jit(
                shard_map(_make_body(reps), mesh=mesh, in_specs=in_specs,
                          out_specs=out_specs, check_rep=False),
                keep_unused=True,
            )

        self._fns = {}
        self._jit = _jit
        self._fn = self.get_fn(1)
        self._zero_concat = [
            np.zeros((n_cores * z.shape[0], *z.shape[1:]), z.dtype) for z in zero_outs
        ]

    def run(self, in_maps):
        concat_in = [
            np.concatenate([np.asarray(m[name]) for m in in_maps], axis=0)
            for name in self.in_names
        ]
        t0 = time.time()
        out_arrs = self._fn(*concat_in, *self._zero_concat)
        out_arrs = [np.asarray(a) for a in out_arrs]
        LAST_RUN["run_s"] = time.time() - t0
        return [
            {
                name: out_arrs[i].reshape(self.n_cores, *self.out_avals[i].shape)[c]
                for i, name in enumerate(self.out_names)
            }
            for c in range(self.n_cores)
        ]

    def get_fn(self, reps):
        if reps not in self._fns:
            self._fns[reps] = self._jit(reps)
        return self._fns[reps]


def _route(residual: np.ndarray, W_router: np.ndarray):
    """Host router: softmax over experts, top-2 (desc, ties -> lower idx),
    renormalize. Returns per-expert (token_ids, weights)."""
    X = residual.reshape(N, D).astype(np.float32)
    logits = X @ W_router.astype(np.float32)
    mx = logits.max(axis=-1, keepdims=True)
    e = np.exp(logits - mx)
    probs = e / e.sum(axis=-1, keepdims=True)
    order = np.argsort(-probs, axis=-1, kind="stable")[:, :TOP_K]       # [N, 2]
    vals = np.take_along_axis(probs, order, axis=-1)                     # [N, 2]
    wts = vals / (vals.sum(axis=-1, keepdims=True) + 1e-8)
    ids, ws = [], []
    for ex in range(E):
        hit = order == ex                                                # [N, 2]
        sel = np.nonzero(hit.any(axis=-1))[0]
        w_tok = np.where(hit[sel, 0], wts[sel, 0], wts[sel, 1]).astype(np.float32)
        ids.append(sel)
        ws.append(w_tok)
    return X, ids, ws


def _plan(counts):
    """Choose C and the spill layout.

    Returns (C, spills) where spills[c] = (expert, tok_lo, tok_hi) of the
    overflow slice of `expert`'s token list placed in core c's Y chunk, or
    None. If no expert overflows XCOLS, returns a plain (C_uniform, None).
    """
    mx = max(counts)
    if mx <= XCOLS:
        return max(TCHUNK, ((mx + 31) // 32) * 32), None
    overflows = [(e, n - XCOLS) for e, n in enumerate(counts) if n > XCOLS]
    for Y in range(32, 513, 32):
        need = sum(-(-o // Y) for _, o in overflows)
        if need <= E:
            blocks = []
            for e, o in overflows:
                lo = XCOLS
                while o > 0:
                    take = min(Y, o)
                    blocks.append((e, lo, lo + take))
                    lo += take
                    o -= take
            spills = [None] * E
            # place each expert's own overflow on its own core first
            rest = []
            for b in blocks:
                if spills[b[0]] is None:
                    spills[b[0]] = b
                else:
                    rest.append(b)
            free = [c for c in range(E) if spills[c] is None]
            # fill remaining blocks onto cores with the fewest primary tokens
            free.sort(key=lambda c: counts[c])
            for b, c in zip(rest, free):
                spills[c] = b
            C_spill = XCOLS + Y
            C_uni = max(TCHUNK, ((mx + 31) // 32) * 32)
            if C_spill < C_uni:
                return C_spill, spills
            return C_uni, None
    return max(TCHUNK, ((mx + 31) // 32) * 32), None


def kernel(
    residual, W_router, W_gate, b_gate, W_up, b_up, W_out, b_out
) -> np.ndarray:
    # NOTE: b_gate/b_up/b_out have fill=zeros in the problem spec and are
    # therefore not applied on-device.
    import ml_dtypes

    bf16 = ml_dtypes.bfloat16

    t_host0 = time.time()
    X, ids, ws = _route(np.asarray(residual), np.asarray(W_router))
    counts = [len(s) for s in ids]
    C, spills = _plan(counts)
    Y = C - XCOLS if spills is not None else 0

    X16 = X.astype(bf16)
    W_gate = np.asarray(W_gate, dtype=np.float32).astype(bf16)
    W_up = np.asarray(W_up, dtype=np.float32).astype(bf16)
    W_out = np.asarray(W_out, dtype=np.float32).astype(bf16)

    def wg_layout(w):   # [D, M] -> [MC, P(d), KD, P(m)]
        return np.ascontiguousarray(w.reshape(KD, P, MC, P).transpose(2, 1, 0, 3))

    def wo_layout(w):   # [M, D] -> [DC, P(m), KM, P(d)]
        return np.ascontiguousarray(w.reshape(KM, P, DC, P).transpose(2, 1, 0, 3))

    zg = np.zeros((MC, P, KD, P), bf16)
    zo = np.zeros((DC, P, KM, P), bf16)

    in_maps = []
    col_ids = []       # per-core token-row ids for scatter (X part + Y part)
    for ex in range(E):
        n_x = min(counts[ex], XCOLS)
        xt = np.zeros((P, KD, C), bf16)
        xt[:, :, :n_x] = X16[ids[ex][:n_x]].T.reshape(KD, P, n_x).transpose(1, 0, 2)
        wrep = np.zeros((P, C), np.float32)
        wrep[:, :n_x] = ws[ex][None, :n_x]
        m = {
            "xt": xt,
            "wg": wg_layout(W_gate[ex]),
            "wu": wg_layout(W_up[ex]),
            "wo": wo_layout(W_out[ex]),
            "wrep": wrep,
        }
        cid = [ids[ex][:n_x]]
        if Y:
            sp = spills[ex]
            if sp is not None:
                se, lo, hi = sp
                n_y = hi - lo
                xt[:, :, XCOLS : XCOLS + n_y] = (
                    X16[ids[se][lo:hi]].T.reshape(KD, P, n_y).transpose(1, 0, 2)
                )
                wrep[:, XCOLS : XCOLS + n_y] = ws[se][None, lo:hi]
                m["wg2"] = wg_layout(W_gate[se])
                m["wu2"] = wg_layout(W_up[se])
                m["wo2"] = wo_layout(W_out[se])
                cid.append(ids[se][lo:hi])
            else:
                m["wg2"], m["wu2"], m["wo2"] = zg, zg, zo
        in_maps.append(m)
        col_ids.append(cid)
    LAST_RUN["host_prep_s"] = time.time() - t_host0
    LAST_RUN["C"] = C
    LAST_RUN["counts"] = counts
    LAST_RUN["spills"] = spills
    LAST_RUN["in_maps"] = in_maps

    if C not in _runner_cache:
        t0 = time.time()
        nc = _build_bass(C)
        LAST_RUN["build_s"] = time.time() - t0
        _runner_cache[C] = _Runner(nc)
    runner = _runner_cache[C]
    results = runner.run(in_maps)

    res = np.zeros((N, D), np.float32)
    for ex in range(E):
        y = results[ex]["out"].reshape(D, C)                            # [D, C]
        n_x = len(col_ids[ex][0])
        res[col_ids[ex][0]] += y[:, :n_x].T
        if len(col_ids[ex]) > 1:
            n_y = len(col_ids[ex][1])
            res[col_ids[ex][1]] += y[:, XCOLS : XCOLS + n_y].T
    return res.reshape(B, S, D)


def get_runner(C: int):
    return _runner_cache.get(C)
